# revision 18
# baseline (speedup 1.0000x reference)
"""Mueller-matrix pyramid kernel for Trainium2 (8 NeuronCores).

Sharding: 8 cores = (batch 4) x (H-halves 2). Each core computes the full
51-channel output for its 256-row half at 512 cols.

v2 design (vs the fp32 adjugate baseline):
- fp16 on-chip everywhere: DVE 2-byte tensor_tensor ops run at 2x; the host
  converts inputs to fp16 ([rows, ch, cols] layout so DMA lines are one
  contiguous 48KB run per partition), halving input DMA bytes.
- M = A^-1 I W^-1 via row-normalized Gauss elimination (DVE divide runs at
  the fast rate): ~292 plane-ops/pixel vs ~440 for the adjugate form. The
  right division M W = P is solved as W^T M^T = P^T using transposed plane
  views (free). Batched 4D-AP broadcast products keep instruction counts low.
- Engine split: DVE owns the solves; Pool owns maxpooling + the W-matrix
  internal elimination (independent chain); Act converts fp16->fp32 output
  and drains PSUM; PE does the bilinear upsample in fp16.
- Output written as [rows, 51, 512] fp32 (host transposes back).

SPMD uniformity: 16 halo rows arrive as a separate channels-on-partitions
tensor; per-core R matrices absorb the local->global row permutation.
"""

import numpy as np

H = W = 512
CIN = 48
HALF = 256          # output rows per core
HALO = 16           # extra pooling rows per core
L1R = 68            # local level-1 rows (64 main + 4 halo)
L1W = 128
L2R = 17            # local level-2 rows (16 main + 1 halo)
L2W = 32
PACK2 = 8           # lvl2 packed cols per partition-row (68*8 = 17*32)
FTW = L1W + PACK2   # 136
N_CORES = 8

# ---------------------------------------------------------------------------
# host-side constants
# ---------------------------------------------------------------------------


def _interp_1d(n_out, n_in, lo, hi):
    out = np.zeros((hi - lo, n_in), np.float32)
    scale = (n_in - 1.0) / (n_out - 1.0)
    for i, y in enumerate(range(lo, hi)):
        t = np.float32(y * scale)
        y0 = int(np.floor(t))
        fy = np.float32(t - y0)
        y1 = min(y0 + 1, n_in - 1)
        out[i, y0] += np.float32(1.0) - fy
        out[i, y1] += fy
    return out


def _r_matrix(half, n_in, n_main, off_main, off_halo, n_halo):
    lo, hi = half * HALF, half * HALF + HALF
    full = _interp_1d(H, n_in, lo, hi)
    loc = np.zeros((HALF, n_main + n_halo), np.float32)
    loc[:, :n_main] = full[:, off_main:off_main + n_main]
    loc[:, n_main:] = full[:, off_halo:off_halo + n_halo]
    return loc


def _host_constants(half):
    if half == 0:
        r1 = _r_matrix(0, 128, 64, 0, 64, 4)
        r2 = _r_matrix(0, 32, 16, 0, 16, 1)
    else:
        r1 = _r_matrix(1, 128, 64, 64, 60, 4)
        r2 = _r_matrix(1, 32, 16, 16, 15, 1)
    c1 = _interp_1d(W, L1W, 0, W).T.copy()
    c2 = _interp_1d(W, L2W, 0, W).T.copy()
    return (np.ascontiguousarray(r1.T).astype(np.float16),
            np.ascontiguousarray(r2.T).astype(np.float16),
            np.ascontiguousarray(c1).astype(np.float16),
            np.ascontiguousarray(c2).astype(np.float16))


_NC_CACHE = {}


def _build_nc(repeat=1):
    import concourse.bacc as bacc
    import concourse.mybir as mybir
    from concourse.tile import TileContext
    from concourse.masks import make_identity

    f16 = mybir.dt.float16
    f32 = mybir.dt.float32
    ALU = mybir.AluOpType

    nc = bacc.Bacc("TRN2", target_bir_lowering=False, num_devices=N_CORES)

    xm = nc.dram_tensor("xm", [HALF, CIN, W], f16, kind="ExternalInput")
    xh = nc.dram_tensor("xh", [CIN, HALO, W], f16, kind="ExternalInput")
    r1t = nc.dram_tensor("r1t", [L1R, HALF], f16, kind="ExternalInput")
    r2t = nc.dram_tensor("r2t", [L2R, HALF], f16, kind="ExternalInput")
    c1 = nc.dram_tensor("c1", [L1W, W], f16, kind="ExternalInput")
    c2 = nc.dram_tensor("c2", [L2W, W], f16, kind="ExternalInput")
    out = nc.dram_tensor("out", [HALF, 51, W], f32, kind="ExternalOutput")

    V, G, S = nc.vector, nc.gpsimd, nc.scalar

    def TT(eng, o, a, b, op):
        eng.tensor_tensor(out=o, in0=a, in1=b, op=op)

    def gauss(xB, xBt, xC, sB, sC, sP, sR, rs, FD):
        """Row-normalized Gauss solve of B Y = C, fully in place (Y -> xC).

        xB: [rs,4,4,FD] planes of B; xBt: transposed-index view of the SAME
        planes (xBt[:,v,u] is B[u][v]); xC: [rs,4,4,FD] RHS planes.
        Engine split with ONE-WAY cross-engine flow: DVE runs the B
        elimination, the reciprocals (sR) and RHS columns 0:3; Pool runs RHS
        column 3 end-to-end (it only consumes DVE outputs, never gates it).
        sB/sC: DVE scratch [rs,3,3,FD]; sP: Pool scratch [rs,3,1,FD].
        """
        for k in range(4):
            piv = xB[:, k:k + 1, k:k + 1, :]
            rk = sR[:, k:k + 1, :].unsqueeze(1)
            V.reciprocal(rk, piv)
            m = 3 - k
            if m:
                rowB = xB[:, k:k + 1, k + 1:, :]
                TT(V, rowB, rowB, rk.broadcast_to((rs, 1, m, FD)), ALU.mult)
            rowCv = xC[:, k:k + 1, 0:3, :]
            TT(V, rowCv, rowCv, rk.broadcast_to((rs, 1, 3, FD)), ALU.mult)
            rowCp = xC[:, k:k + 1, 3:4, :]
            TT(G, rowCp, rowCp, rk.broadcast_to((rs, 1, 1, FD)), ALU.mult)
            if m:
                colB = xB[:, k + 1:, k:k + 1, :]
                pB = sB[:, 0:m, 0:m, :]
                TT(V, pB, colB.broadcast_to((rs, m, m, FD)),
                   rowB.broadcast_to((rs, m, m, FD)), ALU.mult)
                TT(V, xB[:, k + 1:, k + 1:, :], xB[:, k + 1:, k + 1:, :],
                   pB, ALU.subtract)
                pC = sC[:, 0:m, :, :]
                TT(V, pC, colB.broadcast_to((rs, m, 3, FD)),
                   rowCv.broadcast_to((rs, m, 3, FD)), ALU.mult)
                TT(V, xC[:, k + 1:, 0:3, :], xC[:, k + 1:, 0:3, :], pC,
                   ALU.subtract)
                pP = sP[:, 0:m, :, :]
                TT(G, pP, colB.broadcast_to((rs, m, 1, FD)),
                   rowCp.broadcast_to((rs, m, 1, FD)), ALU.mult)
                TT(G, xC[:, k + 1:, 3:4, :], xC[:, k + 1:, 3:4, :], pP,
                   ALU.subtract)
        for i in (2, 1, 0):
            m = 3 - i
            urow = xBt[:, i + 1:, i:i + 1, :]
            pC = sC[:, 0:m, :, :]
            TT(V, pC, urow.broadcast_to((rs, m, 3, FD)),
               xC[:, i + 1:, 0:3, :], ALU.mult)
            for j in range(1, m):
                TT(V, pC[:, 0:1], pC[:, 0:1], pC[:, j:j + 1], ALU.add)
            TT(V, xC[:, i:i + 1, 0:3, :], xC[:, i:i + 1, 0:3, :], pC[:, 0:1],
               ALU.subtract)
            pP = sP[:, 0:m, :, :]
            TT(G, pP, urow.broadcast_to((rs, m, 1, FD)),
               xC[:, i + 1:, 3:4, :], ALU.mult)
            for j in range(1, m):
                TT(G, pP[:, 0:1], pP[:, 0:1], pP[:, j:j + 1], ALU.add)
            TT(G, xC[:, i:i + 1, 3:4, :], xC[:, i:i + 1, 3:4, :], pP[:, 0:1],
               ALU.subtract)

    def solve_views(xt, base):
        nat = xt[:, base:base + 16].rearrange("p (i j) w -> p i j w", j=4)
        tr = xt[:, base:base + 16].rearrange("p (i j) w -> p j i w", j=4)
        return nat, tr

    with nc.allow_low_precision(reason="fp16 pipeline; tol 2e-2"), \
            TileContext(nc) as tc:
        with (
            tc.tile_pool(name="cst", bufs=1) as pool_c,
            tc.tile_pool(name="lvl", bufs=1) as pool_l,
        ):
            ident = pool_c.tile([128, 128], f16)
            make_identity(nc, ident)
            r1t_s = pool_c.tile([L1R, HALF], f16)
            nc.sync.dma_start(out=r1t_s, in_=r1t[:, :])
            r2t_s = pool_c.tile([L2R, HALF], f16)
            nc.sync.dma_start(out=r2t_s, in_=r2t[:, :])
            c1_s = pool_c.tile([L1W, W], f16)
            nc.sync.dma_start(out=c1_s, in_=c1[:, :])
            c2_s = pool_c.tile([L2W, W], f16)
            nc.sync.dma_start(out=c2_s, in_=c2[:, :])

            for _rep in range(repeat):
                lvl1 = pool_l.tile([L1R, CIN, FTW], f16)

                with (
                    tc.tile_pool(name="px", bufs=2) as pool_x,
                    tc.tile_pool(name="ps", bufs=1) as pool_s,
                    tc.tile_pool(name="po", bufs=2) as pool_o,
                ):
                    # ---------------- loads (highest priority) -------------
                    xts = []
                    for rt in range(2):
                        xt = pool_x.tile([128, CIN, W], f16, tag="xt")
                        nc.sync.dma_start(
                            out=xt, in_=xm[rt * 128:(rt + 1) * 128, :, :])
                        xts.append(xt)

                    sBt = pool_s.tile([128, 9, W], f16, tag="sB")
                    sPt = pool_s.tile([128, 4, W], f16, tag="sP")
                    sIt = sPt
                    sSum = pool_s.tile([128, 2, W], f16, tag="sSum")
                    sRa = pool_s.tile([128, 4, W], f16, tag="sRa")
                    sRw = pool_s.tile([128, 4, W], f16, tag="sRw")
                    sB = sBt.rearrange("p (a b) w -> p a b w", b=3)
                    sC = sB
                    sP = sPt.rearrange("p (a b) w -> p a b w", b=1)

                    # ------- pools: W-pool, halo, H-pool, lvl2 (scope H) ---
                    with (
                        tc.tile_pool(name="ph", bufs=1) as pool_h,
                        tc.tile_pool(name="p2a", bufs=1) as pool_2a,
                    ):
                        wp = pool_h.tile([128, 2, CIN, L1W], f16, tag="wp")
                        for rt in range(2):
                            xt = xts[rt]
                            xv = xt.rearrange("p c (f q) -> p c f q", q=128)
                            wps = wp[:, rt]
                            TT(V, wps, xv[:, :, 0], xv[:, :, 1], ALU.max)
                            TT(V, wps, wps, xv[:, :, 2], ALU.max)
                            TT(V, wps, wps, xv[:, :, 3], ALU.max)
                            TT(G, sIt[:, 0:4], xt[:, 0:4], xt[:, 4:8],
                               ALU.add)
                            TT(G, sIt[:, 0:4], sIt[:, 0:4], xt[:, 8:12],
                               ALU.add)
                            TT(G, sIt[:, 0:4], sIt[:, 0:4], xt[:, 12:16],
                               ALU.add)
                            TT(G, sIt[:, 0:2], sIt[:, 0:2], sIt[:, 2:4],
                               ALU.add)
                            TT(G, sSum[:, rt], sIt[:, 0], sIt[:, 1], ALU.add)
                        hw_ = pool_h.tile([CIN, HALO, L1W], f16, tag="hw")
                        xhv = xh.rearrange("c r (f q) -> c r f q", q=128)
                        for hh in range(4):
                            qs = slice(hh * 32, hh * 32 + 32)
                            xht = pool_h.tile([CIN, HALO, 4, L1W // 4],
                                              f16, tag="xht", bufs=1)
                            nc.sync.dma_start(out=xht,
                                              in_=xhv[:, :, :, qs])
                            hwh = hw_[:, :, qs]
                            TT(V, hwh, xht[:, :, 0], xht[:, :, 1], ALU.max)
                            TT(V, hwh, hwh, xht[:, :, 2], ALU.max)
                            TT(V, hwh, hwh, xht[:, :, 3], ALU.max)
                        hp = pool_h.tile([CIN, 4, L1W], f16, tag="hp")
                        hv = hw_.rearrange("c (g r) w -> c g r w", r=4)
                        TT(V, hp, hv[:, :, 0], hv[:, :, 1], ALU.max)
                        TT(V, hp, hp, hv[:, :, 2], ALU.max)
                        TT(V, hp, hp, hv[:, :, 3], ALU.max)
                        for g in range(4):
                            nc.sync.dma_start(
                                out=lvl1[64 + g:65 + g, :, 0:L1W],
                                in_=hp[:, g:g + 1, :])

                        # H-pool main rows
                        wpr = wp.rearrange("(r f) t c w -> r f t c w", f=4)
                        for t in range(2):
                            nc.sync.dma_start(
                                out=lvl1[32 * t:32 * t + 32, :, 0:L1W],
                                in_=wpr[:, 0, t])
                        for dy in (1, 2, 3):
                            for ch in range(2):
                                gt = pool_h.tile([64, CIN, L1W // 2], f16,
                                                 tag="gt", bufs=2)
                                wsl = slice(64 * ch, 64 * ch + 64)
                                for t in range(2):
                                    nc.sync.dma_start(
                                        out=gt[32 * t:32 * t + 32],
                                        in_=wpr[:, dy, t, :, wsl])
                                TT(V, lvl1[0:64, :, wsl],
                                   lvl1[0:64, :, wsl], gt, ALU.max)

                        # lvl2 pooling
                        wp2 = pool_2a.tile([L1R, CIN, L2W], f16, tag="wp2")
                        l14 = lvl1[:, :, 0:L1W].rearrange(
                            "p c (w f) -> p c w f", f=4)
                        TT(V, wp2, l14[:, :, :, 0], l14[:, :, :, 1], ALU.max)
                        TT(V, wp2, wp2, l14[:, :, :, 2], ALU.max)
                        TT(V, wp2, wp2, l14[:, :, :, 3], ALU.max)
                        wp2r = wp2.rearrange("(r f) c w -> r f c w", f=4)
                        l2 = pool_2a.tile([L2R, CIN, L2W], f16, tag="l2")
                        nc.sync.dma_start(out=l2, in_=wp2r[:, 0])
                        for dy in (1, 2, 3):
                            g2 = pool_2a.tile([L2R, CIN, L2W], f16,
                                              tag="g2", bufs=1)
                            nc.sync.dma_start(out=g2, in_=wp2r[:, dy])
                            TT(V, l2, l2, g2, ALU.max)
                        lvl1p = lvl1.rearrange("(r f) c w -> r f c w", f=4)
                        for m in range(4):
                            nc.sync.dma_start(
                                out=lvl1p[0:17, m, :, L1W:FTW],
                                in_=l2[:, :, 8 * m:8 * m + 8])

                    # ---------- lvl-solve + upsample (scope U) -------------
                    with (
                        tc.tile_pool(name="p2", bufs=1) as pool_2,
                        tc.tile_pool(name="pup", bufs=1) as pool_up,
                        tc.tile_pool(name="psw", bufs=2,
                                     space="PSUM") as pool_pw,
                        tc.tile_pool(name="psf", bufs=1,
                                     space="PSUM") as pool_pf,
                    ):
                        sBt2 = pool_2.tile([L1R, 9, FTW], f16, tag="sB2")
                        sPt2 = pool_2.tile([L1R, 3, FTW], f16, tag="sP2")
                        sIt2 = pool_2.tile([L1R, 8, FTW], f16, tag="sI2")
                        sSum2 = pool_2.tile([L1R, 1, FTW], f16, tag="sSum2")
                        sRa2 = pool_2.tile([L1R, 4, FTW], f16, tag="sRa2")
                        sRw2 = pool_2.tile([L1R, 4, FTW], f16, tag="sRw2")
                        ot2 = pool_2.tile([L1R, 17, FTW], f16, tag="ot2")
                        sB2 = sBt2.rearrange("p (a b) w -> p a b w", b=3)
                        sC2 = sB2
                        sP2 = sPt2.rearrange("p (a b) w -> p a b w", b=1)

                        TT(G, sIt2[:, 0:8], lvl1[:, 0:8], lvl1[:, 8:16],
                           ALU.add)
                        TT(G, sIt2[:, 0:4], sIt2[:, 0:4], sIt2[:, 4:8],
                           ALU.add)
                        TT(G, sIt2[:, 0:2], sIt2[:, 0:2], sIt2[:, 2:4],
                           ALU.add)
                        TT(G, sSum2[:, 0], sIt2[:, 0], sIt2[:, 1], ALU.add)
                        V.tensor_scalar(out=ot2[:, 0], in0=sSum2[:, 0],
                                        scalar1=1.0 / 16.0, scalar2=None,
                                        op0=ALU.mult)

                        li4, li4t = solve_views(lvl1, 0)
                        la4, la4t = solve_views(lvl1, 16)
                        lw4, lw4t = solve_views(lvl1, 32)
                        gauss(la4, la4t, li4, sB2, sC2, sP2, sRa2, L1R, FTW)
                        gauss(lw4t, lw4, li4t, sB2, sC2, sP2, sRw2, L1R, FTW)
                        V.reciprocal(sRa2[:, 0:1, :], lvl1[:, 0:1])
                        TT(V, ot2[:, 1:17], lvl1[:, 0:16],
                           sRa2[:, 0:1, :].broadcast_to((L1R, 16, FTW)),
                           ALU.mult)

                        l2f = pool_2.tile([L2R, 17, L2W], f16, tag="l2f")
                        ot2p = ot2.rearrange("(r f) k w -> r f k w", f=4)
                        for m in range(4):
                            nc.sync.dma_start(
                                out=l2f[:, :, 8 * m:8 * m + 8],
                                in_=ot2p[0:17, m, :, L1W:FTW])

                        # bilinear upsample via PE; PSUM drains spread over
                        # Act/DVE/Pool
                        cp_engs = [S, V]

                        def upsample(nch_base, rloc, wloc, rts, cs,
                                     plane_fn, lv):
                            groups = [(c, min(2, 17 - c))
                                      for c in range(0, 17, 2)]
                            for goff, (g0, gn) in enumerate(groups):
                                wrs = []
                                for gi in range(gn):
                                    chn = g0 + gi
                                    pst = pool_pw.tile([128, 128], f16,
                                                       tag="pst")
                                    nc.tensor.transpose(
                                        pst[0:wloc, 0:rloc], plane_fn(chn),
                                        ident[0:rloc, 0:rloc])
                                    pts = pool_up.tile([L1W, L1R], f16,
                                                       tag=f"pts{lv}",
                                                       bufs=2)
                                    S.copy(pts[0:wloc, 0:rloc],
                                           pst[0:wloc, 0:rloc])
                                    psw = pool_pw.tile([L1R, W], f32,
                                                       tag="psw")
                                    nc.tensor.matmul(psw[0:rloc, :],
                                                     pts[0:wloc, 0:rloc],
                                                     cs[0:wloc, :],
                                                     start=True, stop=True)
                                    wres = pool_up.tile(
                                        [L1R, W], f16,
                                        tag=f"wres{lv}{gi}", bufs=2)
                                    cpe = cp_engs[(goff + gi) % 2]
                                    if cpe is S:
                                        S.copy(wres[0:rloc, :],
                                               psw[0:rloc, :])
                                    else:
                                        cpe.tensor_copy(out=wres[0:rloc, :],
                                                        in_=psw[0:rloc, :])
                                    wrs.append(wres)
                                psf = pool_pf.tile([128, 2, W], f32,
                                                   tag="psf", bufs=2)
                                for yb in range(2):
                                    for gi in range(gn):
                                        nc.tensor.matmul(
                                            psf[:, gi, :],
                                            rts[0:rloc,
                                                yb * 128:(yb + 1) * 128],
                                            wrs[gi][0:rloc, :],
                                            start=True, stop=True)
                                    fin = pool_up.tile([128, 2, W], f32,
                                                       tag="fin", bufs=2)
                                    cpe = cp_engs[(goff + yb) % 2]
                                    if cpe is S:
                                        S.copy(fin[:, 0:gn], psf[:, 0:gn])
                                    else:
                                        cpe.tensor_copy(out=fin[:, 0:gn],
                                                        in_=psf[:, 0:gn])
                                    nc.sync.dma_start(
                                        out=out[yb * 128:(yb + 1) * 128,
                                                nch_base + g0:
                                                nch_base + g0 + gn, :],
                                        in_=fin[:, 0:gn])

                        upsample(17, L1R, L1W, r1t_s, c1_s,
                                 lambda chn: ot2[0:L1R, chn, 0:L1W], 1)
                        upsample(34, L2R, L2W, r2t_s, c2_s,
                                 lambda chn: l2f[0:L2R, chn, 0:L2W], 2)

                    # -------------- tile solves (emit last) ----------------
                    for rt in range(2):
                        xt = xts[rt]
                        xi4, xi4t = solve_views(xt, 0)
                        xa4, xa4t = solve_views(xt, 16)
                        xw4, xw4t = solve_views(xt, 32)
                        gauss(xa4, xa4t, xi4, sB, sC, sP, sRa, 128, W)
                        gauss(xw4t, xw4, xi4t, sB, sC, sP, sRw, 128, W)
                        V.reciprocal(sRa[:, 0:1, :], xt[:, 0:1])
                        TT(V, xt[:, 16:32], xt[:, 0:16],
                           sRa[:, 0:1, :].broadcast_to((128, 16, W)),
                           ALU.mult)
                        sSv = sSum[:, rt].rearrange("p (f q) -> p f q",
                                                    q=128)

                        mmv = xt[:, 16:32].rearrange("p c (f q) -> p c f q",
                                                     q=128)
                        for cw in range(4):
                            qs = slice(cw * 32, cw * 32 + 32)
                            otf = pool_o.tile([128, 17, W // 4], f32,
                                              tag="otf")
                            V.tensor_scalar(
                                out=otf[:, 0].rearrange("p (q f) -> p f q",
                                                        f=4),
                                in0=sSv[:, :, qs],
                                scalar1=1.0 / 16.0, scalar2=None,
                                op0=ALU.mult)
                            S.copy(otf[:, 1:17].rearrange(
                                "p c (q f) -> p c f q", f=4),
                                mmv[:, :, :, qs])
                            nc.sync.dma_start(
                                out=out[rt * 128:(rt + 1) * 128, 0:17,
                                        cw * 128:cw * 128 + 128],
                                in_=otf)

    nc.compile()
    return nc


def kernel(x: np.ndarray) -> np.ndarray:
    from concourse.bass_utils import run_bass_kernel_spmd

    assert x.shape == (4, CIN, H, W), x.shape
    x16 = np.ascontiguousarray(x, dtype=np.float32).astype(np.float16)
    B = x.shape[0]

    if "nc" not in _NC_CACHE:
        _NC_CACHE["nc"] = _build_nc()
    nc = _NC_CACHE["nc"]

    consts = [_host_constants(0), _host_constants(1)]
    in_maps = []
    for core in range(N_CORES):
        b, half = core // 2, core % 2
        r1tc, r2tc, c1c, c2c = consts[half]
        r0 = half * HALF
        xmv = x16[b, :, r0:r0 + HALF, :].transpose(1, 0, 2)
        xmv = np.ascontiguousarray(
            xmv.reshape(HALF, CIN, L1W, 4).transpose(0, 1, 3, 2)
            .reshape(HALF, CIN, W))
        if half == 0:
            xhv = x16[b, :, HALF:HALF + HALO, :]
        else:
            xhv = x16[b, :, HALF - HALO:HALF, :]
        xhv = np.ascontiguousarray(
            xhv.reshape(CIN, HALO, L1W, 4).transpose(0, 1, 3, 2)
            .reshape(CIN, HALO, W))
        in_maps.append({
            "xm": xmv, "xh": xhv,
            "r1t": r1tc, "r2t": r2tc, "c1": c1c, "c2": c2c,
        })

    res = run_bass_kernel_spmd(nc, in_maps, core_ids=list(range(N_CORES)))
    outv = np.empty((B, 17 * 3, H, W), np.float32)
    for core in range(N_CORES):
        b, half = core // 2, core % 2
        outv[b, :, half * HALF:(half + 1) * HALF, :] = \
            res.results[core]["out"].transpose(1, 0, 2)
    return outv


# revision 19
# speedup vs baseline: 1.0613x; 1.0613x over previous
"""Mueller-matrix pyramid kernel for Trainium2 (8 NeuronCores).

Sharding: 8 cores = (batch 4) x (H-halves 2). Each core computes the full
51-channel output for its 256-row half at 512 cols.

v2 design (vs the fp32 adjugate baseline):
- fp16 on-chip everywhere: DVE 2-byte tensor_tensor ops run at 2x; the host
  converts inputs to fp16 ([rows, ch, cols] layout so DMA lines are one
  contiguous 48KB run per partition), halving input DMA bytes.
- M = A^-1 I W^-1 via row-normalized Gauss elimination (DVE divide runs at
  the fast rate): ~292 plane-ops/pixel vs ~440 for the adjugate form. The
  right division M W = P is solved as W^T M^T = P^T using transposed plane
  views (free). Batched 4D-AP broadcast products keep instruction counts low.
- Engine split: DVE owns the solves; Pool owns maxpooling + the W-matrix
  internal elimination (independent chain); Act converts fp16->fp32 output
  and drains PSUM; PE does the bilinear upsample in fp16.
- Output written as [rows, 51, 512] fp32 (host transposes back).

SPMD uniformity: 16 halo rows arrive as a separate channels-on-partitions
tensor; per-core R matrices absorb the local->global row permutation.
"""

import numpy as np

H = W = 512
CIN = 48
HALF = 256          # output rows per core
HALO = 16           # extra pooling rows per core
L1R = 68            # local level-1 rows (64 main + 4 halo)
L1W = 128
L2R = 17            # local level-2 rows (16 main + 1 halo)
L2W = 32
PACK2 = 8           # lvl2 packed cols per partition-row (68*8 = 17*32)
FTW = L1W + PACK2   # 136
N_CORES = 8

# ---------------------------------------------------------------------------
# host-side constants
# ---------------------------------------------------------------------------


def _interp_1d(n_out, n_in, lo, hi):
    out = np.zeros((hi - lo, n_in), np.float32)
    scale = (n_in - 1.0) / (n_out - 1.0)
    for i, y in enumerate(range(lo, hi)):
        t = np.float32(y * scale)
        y0 = int(np.floor(t))
        fy = np.float32(t - y0)
        y1 = min(y0 + 1, n_in - 1)
        out[i, y0] += np.float32(1.0) - fy
        out[i, y1] += fy
    return out


def _r_matrix(half, n_in, n_main, off_main, off_halo, n_halo):
    lo, hi = half * HALF, half * HALF + HALF
    full = _interp_1d(H, n_in, lo, hi)
    loc = np.zeros((HALF, n_main + n_halo), np.float32)
    loc[:, :n_main] = full[:, off_main:off_main + n_main]
    loc[:, n_main:] = full[:, off_halo:off_halo + n_halo]
    return loc


def _host_constants(half):
    if half == 0:
        r1 = _r_matrix(0, 128, 64, 0, 64, 4)
        r2 = _r_matrix(0, 32, 16, 0, 16, 1)
    else:
        r1 = _r_matrix(1, 128, 64, 64, 60, 4)
        r2 = _r_matrix(1, 32, 16, 16, 15, 1)
    c1 = _interp_1d(W, L1W, 0, W).T.copy()
    c2 = _interp_1d(W, L2W, 0, W).T.copy()
    return (np.ascontiguousarray(r1.T).astype(np.float16),
            np.ascontiguousarray(r2.T).astype(np.float16),
            np.ascontiguousarray(c1).astype(np.float16),
            np.ascontiguousarray(c2).astype(np.float16))


_NC_CACHE = {}


def _build_nc(repeat=1):
    import concourse.bacc as bacc
    import concourse.mybir as mybir
    from concourse.tile import TileContext
    from concourse.masks import make_identity

    f16 = mybir.dt.float16
    f32 = mybir.dt.float32
    ALU = mybir.AluOpType

    nc = bacc.Bacc("TRN2", target_bir_lowering=False, num_devices=N_CORES)

    xm = nc.dram_tensor("xm", [HALF, CIN, W], f16, kind="ExternalInput")
    xh = nc.dram_tensor("xh", [CIN, HALO, W], f16, kind="ExternalInput")
    r1t = nc.dram_tensor("r1t", [L1R, HALF], f16, kind="ExternalInput")
    r2t = nc.dram_tensor("r2t", [L2R, HALF], f16, kind="ExternalInput")
    c1 = nc.dram_tensor("c1", [L1W, W], f16, kind="ExternalInput")
    c2 = nc.dram_tensor("c2", [L2W, W], f16, kind="ExternalInput")
    out = nc.dram_tensor("out", [HALF, 51, W], f32, kind="ExternalOutput")

    V, G, S = nc.vector, nc.gpsimd, nc.scalar

    def TT(eng, o, a, b, op):
        eng.tensor_tensor(out=o, in0=a, in1=b, op=op)

    def gauss(xB, xBt, xC, sB, sC, sP, sR, rs, FD):
        """Row-normalized Gauss solve of B Y = C, fully in place (Y -> xC).

        xB: [rs,4,4,FD] planes of B; xBt: transposed-index view of the SAME
        planes (xBt[:,v,u] is B[u][v]); xC: [rs,4,4,FD] RHS planes.
        Engine split with ONE-WAY cross-engine flow: DVE runs the B
        elimination, the reciprocals (sR) and RHS columns 0:3; Pool runs RHS
        column 3 end-to-end (it only consumes DVE outputs, never gates it).
        sB/sC: DVE scratch [rs,3,3,FD]; sP: Pool scratch [rs,3,1,FD].
        """
        for k in range(4):
            piv = xB[:, k:k + 1, k:k + 1, :]
            rk = sR[:, k:k + 1, :].unsqueeze(1)
            V.reciprocal(rk, piv)
            m = 3 - k
            if m:
                rowB = xB[:, k:k + 1, k + 1:, :]
                TT(V, rowB, rowB, rk.broadcast_to((rs, 1, m, FD)), ALU.mult)
            rowCv = xC[:, k:k + 1, 0:3, :]
            TT(V, rowCv, rowCv, rk.broadcast_to((rs, 1, 3, FD)), ALU.mult)
            rowCp = xC[:, k:k + 1, 3:4, :]
            TT(G, rowCp, rowCp, rk.broadcast_to((rs, 1, 1, FD)), ALU.mult)
            if m:
                colB = xB[:, k + 1:, k:k + 1, :]
                pB = sB[:, 0:m, 0:m, :]
                TT(V, pB, colB.broadcast_to((rs, m, m, FD)),
                   rowB.broadcast_to((rs, m, m, FD)), ALU.mult)
                TT(V, xB[:, k + 1:, k + 1:, :], xB[:, k + 1:, k + 1:, :],
                   pB, ALU.subtract)
                pC = sC[:, 0:m, :, :]
                TT(V, pC, colB.broadcast_to((rs, m, 3, FD)),
                   rowCv.broadcast_to((rs, m, 3, FD)), ALU.mult)
                TT(V, xC[:, k + 1:, 0:3, :], xC[:, k + 1:, 0:3, :], pC,
                   ALU.subtract)
                pP = sP[:, 0:m, :, :]
                TT(G, pP, colB.broadcast_to((rs, m, 1, FD)),
                   rowCp.broadcast_to((rs, m, 1, FD)), ALU.mult)
                TT(G, xC[:, k + 1:, 3:4, :], xC[:, k + 1:, 3:4, :], pP,
                   ALU.subtract)
        for i in (2, 1, 0):
            m = 3 - i
            urow = xBt[:, i + 1:, i:i + 1, :]
            pC = sC[:, 0:m, :, :]
            TT(V, pC, urow.broadcast_to((rs, m, 3, FD)),
               xC[:, i + 1:, 0:3, :], ALU.mult)
            for j in range(1, m):
                TT(V, pC[:, 0:1], pC[:, 0:1], pC[:, j:j + 1], ALU.add)
            TT(V, xC[:, i:i + 1, 0:3, :], xC[:, i:i + 1, 0:3, :], pC[:, 0:1],
               ALU.subtract)
            pP = sP[:, 0:m, :, :]
            TT(G, pP, urow.broadcast_to((rs, m, 1, FD)),
               xC[:, i + 1:, 3:4, :], ALU.mult)
            for j in range(1, m):
                TT(G, pP[:, 0:1], pP[:, 0:1], pP[:, j:j + 1], ALU.add)
            TT(G, xC[:, i:i + 1, 3:4, :], xC[:, i:i + 1, 3:4, :], pP[:, 0:1],
               ALU.subtract)

    def solve_views(xt, base):
        nat = xt[:, base:base + 16].rearrange("p (i j) w -> p i j w", j=4)
        tr = xt[:, base:base + 16].rearrange("p (i j) w -> p j i w", j=4)
        return nat, tr

    with nc.allow_low_precision(reason="fp16 pipeline; tol 2e-2"), \
            TileContext(nc) as tc:
        with (
            tc.tile_pool(name="cst", bufs=1) as pool_c,
            tc.tile_pool(name="lvl", bufs=1) as pool_l,
        ):
            ident = pool_c.tile([128, 128], f16)
            make_identity(nc, ident)
            r1t_s = pool_c.tile([L1R, HALF], f16)
            nc.sync.dma_start(out=r1t_s, in_=r1t[:, :])
            r2t_s = pool_c.tile([L2R, HALF], f16)
            nc.sync.dma_start(out=r2t_s, in_=r2t[:, :])
            c1_s = pool_c.tile([L1W, W], f16)
            nc.sync.dma_start(out=c1_s, in_=c1[:, :])
            c2_s = pool_c.tile([L2W, W], f16)
            nc.sync.dma_start(out=c2_s, in_=c2[:, :])

            for _rep in range(repeat):
                lvl1 = pool_l.tile([L1R, CIN, FTW], f16)

                with (
                    tc.tile_pool(name="px", bufs=2) as pool_x,
                    tc.tile_pool(name="ps", bufs=1) as pool_s,
                    tc.tile_pool(name="po", bufs=2) as pool_o,
                ):
                    # ---------------- loads (highest priority) -------------
                    xts = []
                    for rt in range(2):
                        xt = pool_x.tile([128, CIN, W], f16, tag="xt")
                        nc.sync.dma_start(
                            out=xt, in_=xm[rt * 128:(rt + 1) * 128, :, :])
                        xts.append(xt)

                    sBt = pool_s.tile([128, 9, W], f16, tag="sB")
                    sPt = pool_s.tile([128, 4, W], f16, tag="sP")
                    sIt = sPt
                    sSum = pool_s.tile([128, 2, W], f16, tag="sSum")
                    sRa = pool_s.tile([128, 4, W], f16, tag="sRa")
                    sRw = pool_s.tile([128, 4, W], f16, tag="sRw")
                    sB = sBt.rearrange("p (a b) w -> p a b w", b=3)
                    sC = sB
                    sP = sPt.rearrange("p (a b) w -> p a b w", b=1)

                    # ------- pools: W-pool, halo, H-pool, lvl2 (scope H) ---
                    with (
                        tc.tile_pool(name="ph", bufs=1) as pool_h,
                        tc.tile_pool(name="p2a", bufs=1) as pool_2a,
                    ):
                        wp = pool_h.tile([128, 2, CIN, L1W], f16, tag="wp")
                        for rt in range(2):
                            xt = xts[rt]
                            xv = xt.rearrange("p c (f q) -> p c f q", q=128)
                            wps = wp[:, rt]
                            TT(V, wps, xv[:, :, 0], xv[:, :, 1], ALU.max)
                            TT(V, wps, wps, xv[:, :, 2], ALU.max)
                            TT(V, wps, wps, xv[:, :, 3], ALU.max)
                            TT(G, sIt[:, 0:4], xt[:, 0:4], xt[:, 4:8],
                               ALU.add)
                            TT(G, sIt[:, 0:4], sIt[:, 0:4], xt[:, 8:12],
                               ALU.add)
                            TT(G, sIt[:, 0:4], sIt[:, 0:4], xt[:, 12:16],
                               ALU.add)
                            TT(G, sIt[:, 0:2], sIt[:, 0:2], sIt[:, 2:4],
                               ALU.add)
                            TT(G, sSum[:, rt], sIt[:, 0], sIt[:, 1], ALU.add)
                        hw_ = pool_h.tile([CIN, HALO, L1W], f16, tag="hw")
                        xhv = xh.rearrange("c r (f q) -> c r f q", q=128)
                        for hh in range(4):
                            qs = slice(hh * 32, hh * 32 + 32)
                            xht = pool_h.tile([CIN, HALO, 4, L1W // 4],
                                              f16, tag="xht", bufs=1)
                            nc.sync.dma_start(out=xht,
                                              in_=xhv[:, :, :, qs])
                            hwh = hw_[:, :, qs]
                            TT(V, hwh, xht[:, :, 0], xht[:, :, 1], ALU.max)
                            TT(V, hwh, hwh, xht[:, :, 2], ALU.max)
                            TT(V, hwh, hwh, xht[:, :, 3], ALU.max)
                        hp = pool_h.tile([CIN, 4, L1W], f16, tag="hp")
                        hv = hw_.rearrange("c (g r) w -> c g r w", r=4)
                        TT(V, hp, hv[:, :, 0], hv[:, :, 1], ALU.max)
                        TT(V, hp, hp, hv[:, :, 2], ALU.max)
                        TT(V, hp, hp, hv[:, :, 3], ALU.max)
                        for g in range(4):
                            nc.sync.dma_start(
                                out=lvl1[64 + g:65 + g, :, 0:L1W],
                                in_=hp[:, g:g + 1, :])

                        # H-pool main rows
                        wpr = wp.rearrange("(r f) t c w -> r f t c w", f=4)
                        for t in range(2):
                            nc.sync.dma_start(
                                out=lvl1[32 * t:32 * t + 32, :, 0:L1W],
                                in_=wpr[:, 0, t])
                        for dy in (1, 2, 3):
                            for ch in range(2):
                                gt = pool_h.tile([64, CIN, L1W // 2], f16,
                                                 tag="gt", bufs=2)
                                wsl = slice(64 * ch, 64 * ch + 64)
                                for t in range(2):
                                    nc.sync.dma_start(
                                        out=gt[32 * t:32 * t + 32],
                                        in_=wpr[:, dy, t, :, wsl])
                                TT(V, lvl1[0:64, :, wsl],
                                   lvl1[0:64, :, wsl], gt, ALU.max)

                        # lvl2 pooling
                        wp2 = pool_2a.tile([L1R, CIN, L2W], f16, tag="wp2")
                        l14 = lvl1[:, :, 0:L1W].rearrange(
                            "p c (w f) -> p c w f", f=4)
                        TT(V, wp2, l14[:, :, :, 0], l14[:, :, :, 1], ALU.max)
                        TT(V, wp2, wp2, l14[:, :, :, 2], ALU.max)
                        TT(V, wp2, wp2, l14[:, :, :, 3], ALU.max)
                        wp2r = wp2.rearrange("(r f) c w -> r f c w", f=4)
                        l2 = pool_2a.tile([L2R, CIN, L2W], f16, tag="l2")
                        nc.sync.dma_start(out=l2, in_=wp2r[:, 0])
                        for dy in (1, 2, 3):
                            g2 = pool_2a.tile([L2R, CIN, L2W], f16,
                                              tag="g2", bufs=1)
                            nc.sync.dma_start(out=g2, in_=wp2r[:, dy])
                            TT(V, l2, l2, g2, ALU.max)
                        lvl1p = lvl1.rearrange("(r f) c w -> r f c w", f=4)
                        for m in range(4):
                            nc.sync.dma_start(
                                out=lvl1p[0:17, m, :, L1W:FTW],
                                in_=l2[:, :, 8 * m:8 * m + 8])

                    # ---------- lvl-solve + upsample (scope U) -------------
                    with (
                        tc.tile_pool(name="p2", bufs=1) as pool_2,
                        tc.tile_pool(name="pup", bufs=1) as pool_up,
                        tc.tile_pool(name="psw", bufs=2,
                                     space="PSUM") as pool_pw,
                        tc.tile_pool(name="psf", bufs=1,
                                     space="PSUM") as pool_pf,
                    ):
                        sBt2 = pool_2.tile([L1R, 9, FTW], f16, tag="sB2")
                        sPt2 = pool_2.tile([L1R, 3, FTW], f16, tag="sP2")
                        sIt2 = pool_2.tile([L1R, 8, FTW], f16, tag="sI2")
                        sSum2 = pool_2.tile([L1R, 1, FTW], f16, tag="sSum2")
                        sRa2 = pool_2.tile([L1R, 4, FTW], f16, tag="sRa2")
                        sRw2 = pool_2.tile([L1R, 4, FTW], f16, tag="sRw2")
                        ot2 = pool_2.tile([L1R, 17, FTW], f16, tag="ot2")
                        sB2 = sBt2.rearrange("p (a b) w -> p a b w", b=3)
                        sC2 = sB2
                        sP2 = sPt2.rearrange("p (a b) w -> p a b w", b=1)

                        TT(G, sIt2[:, 0:8], lvl1[:, 0:8], lvl1[:, 8:16],
                           ALU.add)
                        TT(G, sIt2[:, 0:4], sIt2[:, 0:4], sIt2[:, 4:8],
                           ALU.add)
                        TT(G, sIt2[:, 0:2], sIt2[:, 0:2], sIt2[:, 2:4],
                           ALU.add)
                        TT(G, sSum2[:, 0], sIt2[:, 0], sIt2[:, 1], ALU.add)
                        V.tensor_scalar(out=ot2[:, 0], in0=sSum2[:, 0],
                                        scalar1=1.0 / 16.0, scalar2=None,
                                        op0=ALU.mult)

                        li4, li4t = solve_views(lvl1, 0)
                        la4, la4t = solve_views(lvl1, 16)
                        lw4, lw4t = solve_views(lvl1, 32)
                        gauss(la4, la4t, li4, sB2, sC2, sP2, sRa2, L1R, FTW)
                        gauss(lw4t, lw4, li4t, sB2, sC2, sP2, sRw2, L1R, FTW)
                        V.reciprocal(sRa2[:, 0:1, :], lvl1[:, 0:1])
                        TT(V, ot2[:, 1:17], lvl1[:, 0:16],
                           sRa2[:, 0:1, :].broadcast_to((L1R, 16, FTW)),
                           ALU.mult)

                        l2f = pool_2.tile([L2R, 17, L2W], f16, tag="l2f")
                        ot2p = ot2.rearrange("(r f) k w -> r f k w", f=4)
                        for m in range(4):
                            nc.sync.dma_start(
                                out=l2f[:, :, 8 * m:8 * m + 8],
                                in_=ot2p[0:17, m, :, L1W:FTW])

                        # bilinear upsample via PE; PSUM drains spread over
                        # Act/DVE/Pool
                        cp_engs = [S, V]

                        def upsample(nch_base, rloc, wloc, rts, cs,
                                     plane_fn, lv):
                            groups = [(c, min(2, 17 - c))
                                      for c in range(0, 17, 2)]
                            for goff, (g0, gn) in enumerate(groups):
                                wrs = []
                                for gi in range(gn):
                                    chn = g0 + gi
                                    pst = pool_pw.tile([128, 128], f16,
                                                       tag="pst")
                                    nc.tensor.transpose(
                                        pst[0:wloc, 0:rloc], plane_fn(chn),
                                        ident[0:rloc, 0:rloc])
                                    pts = pool_up.tile([L1W, L1R], f16,
                                                       tag=f"pts{lv}",
                                                       bufs=2)
                                    S.copy(pts[0:wloc, 0:rloc],
                                           pst[0:wloc, 0:rloc])
                                    psw = pool_pw.tile([L1R, W], f32,
                                                       tag="psw")
                                    nc.tensor.matmul(psw[0:rloc, :],
                                                     pts[0:wloc, 0:rloc],
                                                     cs[0:wloc, :],
                                                     start=True, stop=True)
                                    wres = pool_up.tile(
                                        [L1R, W], f16,
                                        tag=f"wres{lv}{gi}", bufs=2)
                                    S.copy(wres[0:rloc, :],
                                           psw[0:rloc, :])
                                    wrs.append(wres)
                                psf = pool_pf.tile([128, 2, W], f32,
                                                   tag="psf", bufs=2)
                                for yb in range(2):
                                    for gi in range(gn):
                                        nc.tensor.matmul(
                                            psf[:, gi, :],
                                            rts[0:rloc,
                                                yb * 128:(yb + 1) * 128],
                                            wrs[gi][0:rloc, :],
                                            start=True, stop=True)
                                    fin = pool_up.tile([128, 2, W], f32,
                                                       tag="fin", bufs=3)
                                    S.copy(fin[:, 0:gn], psf[:, 0:gn])
                                    nc.sync.dma_start(
                                        out=out[yb * 128:(yb + 1) * 128,
                                                nch_base + g0:
                                                nch_base + g0 + gn, :],
                                        in_=fin[:, 0:gn])

                        upsample(17, L1R, L1W, r1t_s, c1_s,
                                 lambda chn: ot2[0:L1R, chn, 0:L1W], 1)
                        upsample(34, L2R, L2W, r2t_s, c2_s,
                                 lambda chn: l2f[0:L2R, chn, 0:L2W], 2)

                    # -------------- tile solves (emit last) ----------------
                    for rt in range(2):
                        xt = xts[rt]
                        xi4, xi4t = solve_views(xt, 0)
                        xa4, xa4t = solve_views(xt, 16)
                        xw4, xw4t = solve_views(xt, 32)
                        gauss(xa4, xa4t, xi4, sB, sC, sP, sRa, 128, W)
                        gauss(xw4t, xw4, xi4t, sB, sC, sP, sRw, 128, W)
                        V.reciprocal(sRa[:, 0:1, :], xt[:, 0:1])
                        TT(V, xt[:, 16:32], xt[:, 0:16],
                           sRa[:, 0:1, :].broadcast_to((128, 16, W)),
                           ALU.mult)
                        sSv = sSum[:, rt].rearrange("p (f q) -> p f q",
                                                    q=128)

                        mmv = xt[:, 16:32].rearrange("p c (f q) -> p c f q",
                                                     q=128)
                        for cw in range(4):
                            qs = slice(cw * 32, cw * 32 + 32)
                            otf = pool_o.tile([128, 17, W // 4], f32,
                                              tag="otf")
                            V.tensor_scalar(
                                out=otf[:, 0].rearrange("p (q f) -> p f q",
                                                        f=4),
                                in0=sSv[:, :, qs],
                                scalar1=1.0 / 16.0, scalar2=None,
                                op0=ALU.mult)
                            S.copy(otf[:, 1:17].rearrange(
                                "p c (q f) -> p c f q", f=4),
                                mmv[:, :, :, qs])
                            nc.sync.dma_start(
                                out=out[rt * 128:(rt + 1) * 128, 0:17,
                                        cw * 128:cw * 128 + 128],
                                in_=otf)

    nc.compile()
    return nc


def kernel(x: np.ndarray) -> np.ndarray:
    from concourse.bass_utils import run_bass_kernel_spmd

    assert x.shape == (4, CIN, H, W), x.shape
    x16 = np.ascontiguousarray(x, dtype=np.float32).astype(np.float16)
    B = x.shape[0]

    if "nc" not in _NC_CACHE:
        _NC_CACHE["nc"] = _build_nc()
    nc = _NC_CACHE["nc"]

    consts = [_host_constants(0), _host_constants(1)]
    in_maps = []
    for core in range(N_CORES):
        b, half = core // 2, core % 2
        r1tc, r2tc, c1c, c2c = consts[half]
        r0 = half * HALF
        xmv = x16[b, :, r0:r0 + HALF, :].transpose(1, 0, 2)
        xmv = np.ascontiguousarray(
            xmv.reshape(HALF, CIN, L1W, 4).transpose(0, 1, 3, 2)
            .reshape(HALF, CIN, W))
        if half == 0:
            xhv = x16[b, :, HALF:HALF + HALO, :]
        else:
            xhv = x16[b, :, HALF - HALO:HALF, :]
        xhv = np.ascontiguousarray(
            xhv.reshape(CIN, HALO, L1W, 4).transpose(0, 1, 3, 2)
            .reshape(CIN, HALO, W))
        in_maps.append({
            "xm": xmv, "xh": xhv,
            "r1t": r1tc, "r2t": r2tc, "c1": c1c, "c2": c2c,
        })

    res = run_bass_kernel_spmd(nc, in_maps, core_ids=list(range(N_CORES)))
    outv = np.empty((B, 17 * 3, H, W), np.float32)
    for core in range(N_CORES):
        b, half = core // 2, core % 2
        outv[b, :, half * HALF:(half + 1) * HALF, :] = \
            res.results[core]["out"].transpose(1, 0, 2)
    return outv
